# revision 2
# baseline (speedup 1.0000x reference)
"""Embedding lookup kernel for Trainium2 (8 NeuronCores, data-parallel).

Problem: out[b, c, :] = embed_matrix[x[b, c], :]
  x:            (4, 2048) int   (values in [0, 50257))
  embed_matrix: (50257, 768) float32
  out:          (4, 2048, 768) float32

Sharding: data parallel over the 8192 flattened indices -> 1024 per core.
The table is replicated to every core's DRAM (it is never staged in SBUF;
only the gathered rows move).  Each core:
  1. DMAs its 1024 int32 indices into SBUF as a [128, 8] tile, column-major
     (idx_tile[p, j] = x_shard[j*128 + p]).
  2. Runs 8 indirect-DMA gathers (SWDGE), one per column: HW semantics are
     one offset per partition, so each gather pulls 128 rows (one 768-float
     contiguous row per partition) from the DRAM table into SBUF.
  3. DMAs each gathered [128, 768] tile to the matching contiguous
     [128, 768] slab of the core's [1024, 768] DRAM output shard.
"""

import numpy as np

VOCAB, EMBED = 50257, 768
B, C = 4, 2048
N_CORES = 8
P = 128
PER_CORE = B * C // N_CORES          # 1024 indices per core
IDX_COLS = PER_CORE // P             # 8 gathers of 128 indices each

SBUF_BUFS = 4

_prog_cache: dict = {}


def _build(bufs: int = SBUF_BUFS):
    """Build + compile the per-core Bass program (identical on all cores)."""
    import concourse.bacc as bacc
    import concourse.bass as bass
    import concourse.mybir as mybir
    from concourse.tile import TileContext

    nc = bacc.Bacc(
        "TRN2",
        target_bir_lowering=False,
        debug=False,
        num_devices=N_CORES,
    )

    idx = nc.dram_tensor("idx", [P, IDX_COLS], mybir.dt.int32, kind="ExternalInput")
    table = nc.dram_tensor(
        "table", [VOCAB, EMBED], mybir.dt.float32, kind="ExternalInput"
    )
    out = nc.dram_tensor(
        "out", [PER_CORE, EMBED], mybir.dt.float32, kind="ExternalOutput"
    )

    with TileContext(nc) as tc:
        with tc.tile_pool(name="sbuf", bufs=bufs) as pool:
            idx_tile = pool.tile([P, IDX_COLS], mybir.dt.int32, tag="idx")
            nc.sync.dma_start(out=idx_tile[:], in_=idx.ap())
            for j in range(IDX_COLS):
                g = pool.tile([P, EMBED], mybir.dt.float32, tag="g")
                nc.gpsimd.indirect_dma_start(
                    out=g[:],
                    out_offset=None,
                    in_=table.ap(),
                    in_offset=bass.IndirectOffsetOnAxis(
                        ap=idx_tile[:, j : j + 1], axis=0
                    ),
                )
                nc.sync.dma_start(out=out.ap()[j * P : (j + 1) * P, :], in_=g[:])

    nc.compile()
    return nc


def _get_prog(bufs: int = SBUF_BUFS):
    if bufs not in _prog_cache:
        _prog_cache[bufs] = _build(bufs)
    return _prog_cache[bufs]


def _make_in_maps(x: np.ndarray, embed_matrix: np.ndarray):
    xf = np.asarray(x).reshape(-1).astype(np.int32)
    table = np.ascontiguousarray(np.asarray(embed_matrix, dtype=np.float32))
    assert xf.shape == (B * C,)
    assert table.shape == (VOCAB, EMBED)
    return [
        {
            # column-major: idx[p, j] = shard[j*P + p]
            "idx": np.ascontiguousarray(
                xf[c * PER_CORE : (c + 1) * PER_CORE].reshape(IDX_COLS, P).T
            ),
            "table": table,
        }
        for c in range(N_CORES)
    ]


def _run(x, embed_matrix, bufs: int = SBUF_BUFS, **spmd_kwargs):
    """Run on hardware; returns (full_output, BassKernelResults)."""
    from concourse import bass_utils

    nc = _get_prog(bufs)
    in_maps = _make_in_maps(x, embed_matrix)
    res = bass_utils.run_bass_kernel_spmd(
        nc, in_maps, core_ids=list(range(N_CORES)), **spmd_kwargs
    )
    outs = [res.results[c]["out"] for c in range(N_CORES)]
    full = np.concatenate(outs, axis=0).reshape(B, C, EMBED)
    return full, res


def kernel(x=None, embed_matrix=None) -> np.ndarray:
    full, _ = _run(x, embed_matrix)
    return full
